# revision 16
# baseline (speedup 1.0000x reference)
"""Trainium2 Bass kernel for a 2-layer LSTM decoder with attention and
weight-tied logits (B=32, S=64, T=64, H=512, V=32000), SPMD over 8 cores.

Strategy:
 - Each core runs the full recurrence (replicated; it is strictly serial in
   T, per-step collectives are too slow to shard it), with the vocab dim of
   the logits GEMM sharded 8 ways (embedding shard kept SBUF-resident).
 - Host (numpy) does only layout prep: transposes, gate permutation,
   embedding gather (indexed by the known trg tokens), 0.5 pre-scaling for
   the tanh-based sigmoid, batch flattening. All math runs on device.
 - Matmuls use the "streaming" orientation: activations stationary
   (lhsT [K,32]), weights streamed as rhs; 4-way tile_position col-tiling
   packs the gate outputs [b+32j, 512] so LSTM elementwise ops are
   [128, 128]-shaped (cheap) and sigmoid(x)=0.5*(1+tanh(x/2)) needs only
   tanh => single activation table set (exp_and_others) for the whole run.
 - Attention: scores as a batched cross-product vs srcq=(Wq@src^T) with a
   -30000 off-diagonal additive mask (also carries src_mask), fused
   exp+row-sum on ACT, PE transposes, then ctx as a clean K=2048 matmul
   against src_flat.
 - Emb contribution of gate preactivations precomputed as one big GEMM
   (P0_all, stored fp16 in DRAM, streamed per step and added into PSUM via
   an identity matmul).
 - dtypes: gates/attn/ctx/Wo path fp16 (exp'd scores bf16 for range),
   scores + logits float32r; c-state and softmax statistics fp32.
   Note: the reference recurrence is chaotic (~1.2x/step error
   amplification; a 1e-6 perturbation reaches ~0.2 absolute dec error by
   t=63), so late-step logits of any reduced-precision implementation
   diverge from the fp32 reference; per-step injected error here is
   ~5e-4 (fp16).
"""
import numpy as np

import concourse.bass as bass
import concourse.bacc as bacc
import concourse.mybir as mybir
import concourse.tile as tile
from concourse.bass_utils import run_bass_kernel_spmd

dt = mybir.dt
AF = mybir.ActivationFunctionType
ALU = mybir.AluOpType

B, S, T, H, E, V = 32, 64, 64, 512, 512, 32000
NCORES = 8
VS = V // NCORES           # 4000 vocab per core
NCHUNK = 8                 # logits N-chunks per 4-step group
VCH = VS // NCHUNK         # 500
GROUP = 4                  # steps per logits group (M=128)
MASK_NEG = -30000.0

_BF16 = None
TRACE = False          # set True (e.g. by test.py) to neuron-profile the run
last_exec_ns = None


def _bf16_dtype():
    global _BF16
    if _BF16 is None:
        import ml_dtypes
        _BF16 = np.dtype(ml_dtypes.bfloat16)
    return _BF16


# --------------------------------------------------------------------------
# device program
# --------------------------------------------------------------------------

def build_nc(nsteps=T):
    nc = bacc.Bacc(None)
    f32, bf16, f32r, f16 = dt.float32, dt.bfloat16, dt.float32r, dt.float16

    # ---- DRAM inputs (per core) ----
    d_embT = nc.dram_tensor("embT", [128, 4 * 32 * nsteps], f16, kind="ExternalInput")
    d_w0emb = nc.dram_tensor("w0embT", [128, 4 * 2048], f16, kind="ExternalInput")
    d_w0r = nc.dram_tensor("w0rT", [128, 8 * 2048], f16, kind="ExternalInput")
    d_w1 = nc.dram_tensor("w1T", [128, 8 * 2048], f16, kind="ExternalInput")
    d_wo = nc.dram_tensor("woT", [128, 8 * 512], f16, kind="ExternalInput")
    d_wq = nc.dram_tensor("wqT", [128, 4 * 512], f32r, kind="ExternalInput")
    d_srcT = nc.dram_tensor("srcT", [128, 4 * 2048], f32r, kind="ExternalInput")
    d_srcflat = nc.dram_tensor("srcflat", [128, 16 * 512], f16, kind="ExternalInput")
    d_mask = nc.dram_tensor("maskb", [32, 2048], bf16, kind="ExternalInput")
    d_esT = nc.dram_tensor("esT", [128, 4 * VS], f32r, kind="ExternalInput")
    # consts: [I32 | Sel] as f32, [I32] as bf16
    d_cst = nc.dram_tensor("cst", [128, 96], f32, kind="ExternalInput")
    d_cstb = nc.dram_tensor("cstb", [128, 32], bf16, kind="ExternalInput")
    d_csth = nc.dram_tensor("csth", [128, 32], f16, kind="ExternalInput")
    # initial state
    d_h0T = nc.dram_tensor("h0Ti", [128, 128], f16, kind="ExternalInput")
    d_h1T = nc.dram_tensor("h1Ti", [128, 128], f16, kind="ExternalInput")
    d_outT = nc.dram_tensor("outTi", [128, 128], f16, kind="ExternalInput")
    d_c0 = nc.dram_tensor("c0i", [128, 128], f32, kind="ExternalInput")
    d_c1 = nc.dram_tensor("c1i", [128, 128], f32, kind="ExternalInput")

    d_p0 = nc.dram_tensor("p0all", [32 * nsteps, 2048], f16)  # internal
    d_out = nc.dram_tensor("logits", [32 * nsteps, VS], f32, kind="ExternalOutput")

    with tile.TileContext(nc) as tc:
        with tc.tile_pool(name="wts", bufs=1) as wp, \
             tc.tile_pool(name="work", bufs=1) as wk, \
             tc.tile_pool(name="psum", bufs=1, space="PSUM") as pp:

            # ---- persistent SBUF ----
            w0r = wp.tile([128, 8 * 2048], f16, tag="w0r")
            w1 = wp.tile([128, 8 * 2048], f16, tag="w1")
            wo = wp.tile([128, 8 * 512], f16, tag="wo")
            srcflat = wp.tile([128, 16 * 512], f16, tag="srcflat")
            srcq = wp.tile([128, 4 * 2048], f32r, tag="srcq")
            mask = wp.tile([32, 2048], bf16, tag="mask")
            cI32b = wp.tile([128, 32], bf16, tag="cI32b")
            cI16 = wp.tile([128, 32], f16, tag="cI16")
            cIf32 = wp.tile([128, 32], f32, tag="cIf32")

            nc.sync.dma_start(out=w0r[:], in_=d_w0r[:])
            nc.sync.dma_start(out=w1[:], in_=d_w1[:])
            nc.sync.dma_start(out=wo[:], in_=d_wo[:])
            nc.sync.dma_start(out=srcflat[:], in_=d_srcflat[:])
            nc.sync.dma_start(out=cI32b[:], in_=d_cstb[:])
            nc.sync.dma_start(out=cI16[:], in_=d_csth[:])

            # ---- pre-GEMM temporaries ----
            with tc.tile_pool(name="tmp", bufs=1) as tp:
                cst_f = tp.tile([128, 96], f32, tag="cstf")
                nc.sync.dma_start(out=cst_f[:], in_=d_cst[:])
                nc.vector.tensor_copy(cIf32[:], cst_f[:, 64:96])
                nc.sync.dma_start(out=mask[:], in_=d_mask[:])
                # P0_all = emb_all @ W0emb^T   (both operands fed transposed)
                w0emb = tp.tile([128, 4 * 2048], f16, tag="w0emb")
                nc.sync.dma_start(out=w0emb[:], in_=d_w0emb[:])
                d_embT_v = d_embT[:].rearrange("p (k n) -> p k n", k=4)
                for m in range(nsteps * 32 // 128):    # 16 M-tiles for T=64
                    embt = tp.tile([128, 512], f16, tag="embt", bufs=2)
                    nc.sync.dma_start(
                        out=embt[:].rearrange("p (k n) -> p k n", k=4),
                        in_=d_embT_v[:, :, 128 * m:128 * (m + 1)])
                    stage = tp.tile([128, 2048], f16, tag="p0stage", bufs=2)
                    for nch in range(4):
                        ps = pp.tile([128, 512], f32, tag="gates", bufs=2)
                        for kt in range(4):
                            nc.tensor.matmul(
                                ps[:, :],
                                embt[:, 128 * kt:128 * (kt + 1)],
                                w0emb[:, 2048 * kt + 512 * nch:2048 * kt + 512 * (nch + 1)],
                                start=(kt == 0), stop=(kt == 3))
                        nc.vector.tensor_copy(stage[:, 512 * nch:512 * (nch + 1)], ps[:])
                    nc.sync.dma_start(out=d_p0[128 * m:128 * (m + 1), :], in_=stage[:])

            with tc.tile_pool(name="tmp2", bufs=1) as tp:
                # srcqT = Wq @ src_flat^T
                wqt = tp.tile([128, 4 * 512], f32r, tag="wqt")
                nc.sync.dma_start(out=wqt[:], in_=d_wq[:])
                d_srcT_v = d_srcT[:].rearrange("p (k n) -> p k n", k=4)
                for nch in range(4):
                    sch = tp.tile([128, 4 * 512], f32r, tag="sch", bufs=2)
                    nc.sync.dma_start(
                        out=sch[:].rearrange("p (k n) -> p k n", k=4),
                        in_=d_srcT_v[:, :, 512 * nch:512 * (nch + 1)])
                    for mt in range(4):
                        ps = pp.tile([128, 512], f32, tag="gates", bufs=2)
                        for kt in range(4):
                            nc.tensor.matmul(
                                ps[:, :],
                                wqt[:, 512 * kt + 128 * mt:512 * kt + 128 * (mt + 1)],
                                sch[:, 512 * kt:512 * (kt + 1)],
                                start=(kt == 0), stop=(kt == 3))
                        nc.vector.tensor_copy(
                            srcq[:, 2048 * mt + 512 * nch:2048 * mt + 512 * (nch + 1)],
                            ps[:])



            # ---- recurrent state ----
            c0 = wk.tile([128, 128], f32, tag="c0")
            c1 = wk.tile([128, 128], f32, tag="c1")
            h0Ti = wk.tile([128, 128], f16, tag="h0Ti")
            h1Ti = wk.tile([128, 128], f16, tag="h1Ti")
            outTi = wk.tile([128, 128], f16, tag="outTi")
            nc.sync.dma_start(out=c0[:], in_=d_c0[:])
            nc.sync.dma_start(out=c1[:], in_=d_c1[:])
            nc.sync.dma_start(out=h0Ti[:], in_=d_h0T[:])
            nc.sync.dma_start(out=h1Ti[:], in_=d_h1T[:])
            nc.sync.dma_start(out=outTi[:], in_=d_outT[:])

            h0T_prev, h1T_prev, outT_prev = h0Ti, h1Ti, outTi

            decTg = wk.tile([128, 4 * 128], f32r, tag="decTg", bufs=2)

            def transpose4(src_ap_fn, outs, psum_tag, bases=(0, 32, 64, 96)):
                """4x PE transpose of [32,128] slices -> copies into outs.

                src_ap_fn(k) -> [32, 128] SBUF AP; outs = list of
                (tile, col_offset, width32) receiving [128, 32] at col k.
                """
                for k in range(4):
                    pt = pp.tile([128, 32], f32, tag=psum_tag, bufs=2)
                    src = src_ap_fn(k)
                    bp = bases[k]
                    nc.tensor.transpose(pt[:, :], src, cIf32[bp:bp + 32, :],
                                        tile_position=(bp, 0))
                    for (otile, base) in outs:
                        nc.vector.tensor_copy(
                            otile[:, base + 32 * k:base + 32 * (k + 1)], pt[:, :])

            for t in range(nsteps):
                # ---- P0 chunk ----
                p0t = wk.tile([32, 2048], f16, tag="p0t", bufs=2)
                nc.sync.dma_start(out=p0t[:], in_=d_p0[32 * t:32 * (t + 1), :])

                # ---- LSTM cell 0 ----
                g0 = pp.tile([128, 512], f32, tag="gates", bufs=2)
                for j in range(4):
                    nc.tensor.matmul(g0[32 * j:32 * (j + 1), :], cI16[0:32, :],
                                     p0t[:, 512 * j:512 * (j + 1)],
                                     start=True, stop=False,
                                     tile_position=(0, 32 * j))
                    for kt in range(8):
                        lhs = (outT_prev[:, 32 * kt:32 * (kt + 1)] if kt < 4 else
                               h0T_prev[:, 32 * (kt - 4):32 * (kt - 3)])
                        nc.tensor.matmul(g0[32 * j:32 * (j + 1), :], lhs,
                                         w0r[:, 2048 * kt + 512 * j:2048 * kt + 512 * (j + 1)],
                                         start=False, stop=(kt == 7),
                                         tile_position=(0, 32 * j))
                T0 = wk.tile([128, 512], f32, tag="T")
                nc.scalar.activation(T0[:], g0[:], AF.Tanh)
                Tp0 = wk.tile([128, 384], f32, tag="Tp")
                nc.vector.tensor_scalar(Tp0[:, 0:256], T0[:, 0:256], 0.5, 0.5,
                                        ALU.mult, ALU.add)
                nc.vector.tensor_scalar(Tp0[:, 256:384], T0[:, 384:512], 0.5, 0.5,
                                        ALU.mult, ALU.add)
                tm1 = wk.tile([128, 128], f32, tag="tm1")
                tm2 = wk.tile([128, 128], f32, tag="tm2")
                nc.vector.tensor_mul(tm1[:], Tp0[:, 128:256], c0[:])
                nc.vector.tensor_mul(tm2[:], Tp0[:, 0:128], T0[:, 256:384])
                nc.vector.tensor_add(c0[:], tm1[:], tm2[:])
                th0 = wk.tile([128, 128], f32, tag="th")
                nc.scalar.activation(th0[:], c0[:], AF.Tanh)
                h0p = wk.tile([128, 128], f32, tag="hp")
                nc.vector.tensor_mul(h0p[:], Tp0[:, 256:384], th0[:])

                h0T = wk.tile([128, 128], f16, tag="h0T", bufs=2)
                transpose4(lambda k: h0p[32 * k:32 * (k + 1), :], [(h0T, 0)], "tp")

                # ---- LSTM cell 1 ----
                g1 = pp.tile([128, 512], f32, tag="gates", bufs=2)
                for j in range(4):
                    for kt in range(8):
                        lhs = (h0T[:, 32 * kt:32 * (kt + 1)] if kt < 4 else
                               h1T_prev[:, 32 * (kt - 4):32 * (kt - 3)])
                        nc.tensor.matmul(g1[32 * j:32 * (j + 1), :], lhs,
                                         w1[:, 2048 * kt + 512 * j:2048 * kt + 512 * (j + 1)],
                                         start=(kt == 0), stop=(kt == 7),
                                         tile_position=(0, 32 * j))
                T1 = wk.tile([128, 512], f32, tag="T")
                nc.scalar.activation(T1[:], g1[:], AF.Tanh)
                Tp1 = wk.tile([128, 384], f32, tag="Tp")
                nc.vector.tensor_scalar(Tp1[:, 0:256], T1[:, 0:256], 0.5, 0.5,
                                        ALU.mult, ALU.add)
                nc.vector.tensor_scalar(Tp1[:, 256:384], T1[:, 384:512], 0.5, 0.5,
                                        ALU.mult, ALU.add)
                nc.vector.tensor_mul(tm1[:], Tp1[:, 128:256], c1[:])
                nc.vector.tensor_mul(tm2[:], Tp1[:, 0:128], T1[:, 256:384])
                nc.vector.tensor_add(c1[:], tm1[:], tm2[:])
                th1 = wk.tile([128, 128], f32, tag="th")
                nc.scalar.activation(th1[:], c1[:], AF.Tanh)
                h1p = wk.tile([128, 128], f32, tag="hp")
                nc.vector.tensor_mul(h1p[:], Tp1[:, 256:384], th1[:])

                h1T = wk.tile([128, 128], f16, tag="h1T", bufs=2)
                h1Tr = wk.tile([128, 128], f32r, tag="h1Tr", bufs=2)
                transpose4(lambda k: h1p[32 * k:32 * (k + 1), :],
                           [(h1T, 0), (h1Tr, 0)], "tp")

                # ---- attention ----
                expS = wk.tile([32, 2048], bf16, tag="expS")
                accums = wk.tile([32, 8], f32, tag="accums")
                for ch in range(4):
                    ssc = pp.tile([32, 512], f32, tag="small", bufs=2)
                    nc.tensor.matmul(ssc[:, :], cI32b[0:32, :],
                                     mask[:, 512 * ch:512 * (ch + 1)],
                                     start=True, stop=False)
                    for kt in range(4):
                        nc.tensor.matmul(ssc[:, :], h1Tr[:, 32 * kt:32 * (kt + 1)],
                                         srcq[:, 2048 * kt + 512 * ch:2048 * kt + 512 * (ch + 1)],
                                         start=False, stop=(kt == 3))
                    nc.scalar.activation(expS[:, 512 * ch:512 * (ch + 1)], ssc[:, :],
                                         AF.Exp, accum_out=accums[:, ch:ch + 1])
                ssum = wk.tile([32, 8], f32, tag="ssum")
                nc.vector.tensor_reduce(ssum[:, 0:1], accums[:, 0:4],
                                        mybir.AxisListType.X, ALU.add)
                recip = wk.tile([32, 1], f32, tag="recip")
                nc.vector.reciprocal(recip[:], ssum[:, 0:1])
                attnN = wk.tile([32, 2048], f16, tag="attnN")
                nc.vector.tensor_scalar(attnN[:], expS[:], recip[:, 0:1], None,
                                        ALU.mult)

                expT = wk.tile([128, 512], f16, tag="expT")
                for k in range(16):
                    ptb = pp.tile([128, 32], f16, tag="tpb", bufs=1)
                    nc.tensor.transpose(ptb[:, :], attnN[:, 128 * k:128 * (k + 1)],
                                        cI16[0:32, :])
                    nc.vector.tensor_copy(expT[:, 32 * k:32 * (k + 1)], ptb[:, :])

                ctxps = pp.tile([32, 512], f32, tag="small", bufs=2)
                for kt in range(16):
                    nc.tensor.matmul(ctxps[:, :], expT[:, 32 * kt:32 * (kt + 1)],
                                     srcflat[:, 512 * kt:512 * (kt + 1)],
                                     start=(kt == 0), stop=(kt == 15))
                ctx = wk.tile([32, 512], f32, tag="ctx")
                nc.vector.tensor_copy(ctx[:], ctxps[:])
                ctxT = wk.tile([128, 128], f16, tag="ctxT")
                transpose4(lambda k: ctx[:, 128 * k:128 * (k + 1)], [(ctxT, 0)], "tp",
                           bases=(0, 0, 0, 0))

                # ---- output projection ----
                dps = pp.tile([32, 512], f32, tag="small", bufs=2)
                for kt in range(8):
                    lhs = (h1T[:, 32 * kt:32 * (kt + 1)] if kt < 4 else
                           ctxT[:, 32 * (kt - 4):32 * (kt - 3)])
                    nc.tensor.matmul(dps[:, :], lhs,
                                     wo[:, 512 * kt:512 * (kt + 1)],
                                     start=(kt == 0), stop=(kt == 7))
                dec = wk.tile([32, 512], f32, tag="dec")
                nc.vector.tensor_copy(dec[:], dps[:])
                outT = wk.tile([128, 128], f16, tag="outT", bufs=2)
                gslot = t % GROUP
                for k in range(4):
                    pt = pp.tile([128, 32], f32, tag="tp", bufs=2)
                    nc.tensor.transpose(pt[:, :], dec[:, 128 * k:128 * (k + 1)],
                                        cIf32[0:32, :])
                    nc.vector.tensor_copy(outT[:, 32 * k:32 * (k + 1)], pt[:, :])
                    nc.vector.tensor_copy(
                        decTg[:, 128 * k + 32 * gslot:128 * k + 32 * (gslot + 1)],
                        pt[:, :])

                h0T_prev, h1T_prev, outT_prev = h0T, h1T, outT

                # ---- logits for the completed group ----
                if gslot == GROUP - 1:
                    g = t // GROUP
                    for nch in range(NCHUNK):
                        esp = []
                        for kt in range(4):
                            ep = wk.tile([128, VCH], f32r, tag="esp", bufs=6)
                            nc.sync.dma_start(
                                out=ep[:],
                                in_=d_esT[:, VS * kt + VCH * nch:
                                          VS * kt + VCH * (nch + 1)])
                            esp.append(ep)
                        lps = pp.tile([128, VCH], f32, tag="logits")
                        for kt in range(4):
                            nc.tensor.matmul(
                                lps[:, :], decTg[:, 128 * kt:128 * (kt + 1)],
                                esp[kt][:],
                                start=(kt == 0), stop=(kt == 3))
                        sbl = wk.tile([128, VCH], f32, tag="sbl", bufs=2)
                        nc.vector.tensor_copy(sbl[:, :], lps[:, :])
                        nc.sync.dma_start(
                            out=d_out[128 * g:128 * (g + 1),
                                      VCH * nch:VCH * (nch + 1)],
                            in_=sbl[:, :])
                    decTg = wk.tile([128, 4 * 128], f32r, tag="decTg", bufs=2)

    nc.finalize()
    return nc


# --------------------------------------------------------------------------
# host side
# --------------------------------------------------------------------------

def _perm_and_scale():
    """Gate permutation: new col (512j + 128g + u) = old row (512g + 128j + u),
    with rows of gates i,f,o pre-scaled by 0.5 (tanh-half sigmoid trick)."""
    perm = np.empty(2048, np.int64)
    for j in range(4):
        for g in range(4):
            perm[512 * j + 128 * g:512 * j + 128 * (g + 1)] = \
                np.arange(512 * g + 128 * j, 512 * g + 128 * (j + 1))
    scale = np.ones(2048, np.float32)
    scale[0:512] = 0.5       # i
    scale[512:1024] = 0.5    # f
    scale[1536:2048] = 0.5   # o
    return perm, scale


def _kt_layout(a, ktiles):
    """[K, N] -> [128, ktiles*N] with K-tile kt at cols [N*kt, N*(kt+1))."""
    K, N = a.shape
    assert K == 128 * ktiles
    return np.ascontiguousarray(
        a.reshape(ktiles, 128, N).transpose(1, 0, 2).reshape(128, ktiles * N))


def _pack128(x):
    """[32, 512] -> packed [128, 128]: row 32j+b = x[b, 128j:128(j+1)]."""
    return np.ascontiguousarray(
        np.concatenate([x[:, 128 * j:128 * (j + 1)] for j in range(4)], axis=0))


def _transT(x):
    """[32, 512] -> hT layout [128, 4*32]: col block k = x[:,128k:].T."""
    return np.ascontiguousarray(
        np.concatenate([x[:, 128 * k:128 * (k + 1)].T for k in range(4)], axis=1))


_NC_CACHE = {}


def kernel(embedding_w, Wih0, Whh0, bih0, bhh0, Wih1, Whh1, bih1, bhh1,
           Wq, bq, Wo, bo, src_memory, init_h, init_c, init_out,
           src_mask, trg, nsteps=T):
    bf16 = _bf16_dtype()
    f32 = np.float32
    for b_ in (bih0, bhh0, bih1, bhh1, bq, bo):
        assert np.abs(np.asarray(b_)).max() == 0.0, \
            "nonzero biases not supported by this kernel build"

    perm, gscale = _perm_and_scale()

    trg_use = np.asarray(trg)[:, :nsteps]
    emb_all = np.asarray(embedding_w)[trg_use.T.reshape(-1)]          # [nsteps*B, E]
    embT = np.ascontiguousarray(emb_all.T)                            # [512, 2048]

    W0emb = (np.asarray(Wih0)[:, :E] * gscale[:, None])[perm]         # [2048, 512]
    w0embT = np.ascontiguousarray(W0emb.T)                            # [512, 2048]

    A0 = np.concatenate([np.asarray(Wih0)[:, E:], np.asarray(Whh0)], axis=1)
    A0 = (A0 * gscale[:, None])[perm]                                 # [2048, 1024]
    w0rT = np.ascontiguousarray(A0.T)                                 # [1024, 2048]

    A1 = np.concatenate([np.asarray(Wih1), np.asarray(Whh1)], axis=1)
    A1 = (A1 * gscale[:, None])[perm]
    w1T = np.ascontiguousarray(A1.T)                                  # [1024, 2048]

    woT = np.ascontiguousarray(np.asarray(Wo).T)                      # [1024, 512]
    wqT = np.ascontiguousarray(np.asarray(Wq))                        # [512, 512] (contract over rows)

    src_flat = np.ascontiguousarray(
        np.asarray(src_memory).reshape(B * S, H))                     # [2048, 512]
    srcT = np.ascontiguousarray(src_flat.T)                           # [512, 2048]

    maskb = np.full((B, B * S), MASK_NEG, f32)
    sm = np.asarray(src_mask)
    for b_ in range(B):
        maskb[b_, S * b_:S * (b_ + 1)] = np.where(sm[b_], 0.0, MASK_NEG)

    eye4 = np.tile(np.eye(32, dtype=f32), (4, 1))          # [128, 32]
    cst = np.zeros((128, 96), f32)
    cst[0:32, 0:32] = np.eye(32, dtype=f32)                # I32 (f32r source)
    cst[:, 32:64] = (np.arange(128)[:, None] % 32 ==
                     np.arange(32)[None, :]).astype(f32)   # Sel
    cst[:, 64:96] = eye4                                   # I32 f32 (transpose)
    cstb = eye4.astype(bf16)

    ih = np.asarray(init_h)
    ic = np.asarray(init_c)
    io = np.asarray(init_out)

    base = {
        "embT": _kt_layout(embT, 4).astype(np.float16),
        "w0embT": _kt_layout(w0embT, 4).astype(np.float16),
        "w0rT": _kt_layout(w0rT, 8).astype(np.float16),
        "w1T": _kt_layout(w1T, 8).astype(np.float16),
        "woT": _kt_layout(woT, 8).astype(np.float16),
        "wqT": _kt_layout(wqT, 4).astype(f32),  # fed as f32r bytes
        "srcT": _kt_layout(srcT, 4).astype(f32),
        "srcflat": _kt_layout(src_flat, 16).astype(np.float16),
        "maskb": maskb.astype(bf16),
        "cst": cst,
        "cstb": cstb,
        "csth": eye4.astype(np.float16),
        "h0Ti": _transT(ih[0]).astype(np.float16),
        "h1Ti": _transT(ih[1]).astype(np.float16),
        "outTi": _transT(io).astype(np.float16),
        "c0i": _pack128(ic[0]).astype(f32),
        "c1i": _pack128(ic[1]).astype(f32),
    }
    ew = np.asarray(embedding_w)
    in_maps = []
    for c in range(NCORES):
        m = dict(base)
        esT = np.ascontiguousarray(ew[VS * c:VS * (c + 1)].T)  # [512, VS]
        m["esT"] = _kt_layout(esT, 4).astype(f32)
        in_maps.append(m)

    if nsteps not in _NC_CACHE:
        _NC_CACHE[nsteps] = build_nc(nsteps)
    nc = _NC_CACHE[nsteps]

    global last_exec_ns
    if TRACE:
        res_list, last_exec_ns = _run_traced(nc, in_maps)
        res = type("R", (), {"results": res_list})()
    else:
        res = run_bass_kernel_spmd(nc, in_maps, list(range(NCORES)))
    out = np.empty((B, nsteps, V), f32)
    for c in range(NCORES):
        oc = res.results[c]["logits"].reshape(nsteps, B, VS)
        out[:, :, VS * c:VS * (c + 1)] = oc.transpose(1, 0, 2)
    return out


# --------------------------------------------------------------------------
# optional NTFF profiling (axon): mirrors trn_boot's ctypes hook
# --------------------------------------------------------------------------

def _run_traced(nc, in_maps):
    import contextlib
    import ctypes
    import glob
    import os
    import tempfile

    from concourse import bass2jax

    if not nc.is_finalized():
        nc.finalize()
    so_path = "/opt/axon/libaxon_pjrt.so"
    try:
        lib = ctypes.CDLL(so_path)
        assert hasattr(lib, "axon_start_nrt_profile")
        lib.axon_start_nrt_profile.argtypes = [
            ctypes.POINTER(ctypes.c_int64), ctypes.c_size_t]
        lib.axon_start_nrt_profile.restype = ctypes.c_int64
        lib.axon_stop_nrt_profile.argtypes = [ctypes.c_char_p]
        lib.axon_stop_nrt_profile.restype = ctypes.c_int64
    except Exception:
        return bass2jax.run_bass_via_pjrt(nc, in_maps, n_cores=NCORES), None

    tmpdir = tempfile.mkdtemp(prefix="ktrace_")
    import jax
    jax.devices()
    ids = (ctypes.c_int64 * 1)(0)
    rc = lib.axon_start_nrt_profile(ids, 1)
    try:
        results = bass2jax.run_bass_via_pjrt(nc, in_maps, n_cores=NCORES)
    finally:
        lib.axon_stop_nrt_profile(str(tmpdir).encode())

    exec_ns = None
    try:
        if glob.glob(os.path.join(tmpdir, "*_body*.ntff")):
            import gauge.profiler
            from concourse._compat import FishPath
            profile = gauge.profiler.Profile(
                profile_path=FishPath(tmpdir), kernel_dev_mode=True,
                profile_on_exit=False, bass_kernel=nc.m,
                offline_processing=True, fname="*_body*")
            prs = profile.to_perfetto(model_index=(0,))
            for pr in prs:
                if pr.exec_time_ns is not None and (
                        exec_ns is None or pr.exec_time_ns > exec_ns):
                    exec_ns = pr.exec_time_ns
            _run_traced.last_perfetto = prs
            _run_traced.last_dir = tmpdir
    except Exception as e:
        print("trace processing failed:", e)
    return results, exec_ns
